# revision 1
# baseline (speedup 1.0000x reference)
"""Trainium2 Bass kernel for KMeans assignment (argmin over 8192 centroids).

Problem: x [32768, 1024] f32, centroids [1024, 8192] f32 ->
         argmin_k ||x_n - c_k||^2  as int32 [32768].

Math: argmin_k (||x||^2 - 2 x.c_k + ||c_k||^2) == argmax_k (x.c_k - 0.5*||c_k||^2).
The ||x||^2 term is row-constant and drops out of the argmin.

Sharding: data-parallel over N across 8 cores (4096 rows each), centroids
replicated. Per core: scores = xT_shard.T @ centroids + bias computed in
16 chunks of 512 centroids; per chunk the DVE max8/max_index ops produce
the chunk top-8 values + indices per row. The 16 chunk winners per row are
merged on the host (trivial numpy argmax over 16 candidates).

Matmul runs in float32r (TF32-like ~11 mantissa bits, 1 cycle/row vs 4 for
fp32). Optional exact refinement: the host re-scores each row's top
candidates in fp32/fp64 to undo f32r rounding on near-ties.
"""
import os
import numpy as np

# ---- problem constants (hardcoded per harness contract) ----
N_FULL, D, K = 32768, 1024, 8192
N_CORES = 8
NC = N_FULL // N_CORES          # 4096 rows per core
NB = 2                          # n-blocks per core
NBLK = NC // NB                 # 2048 rows per block
NT = NBLK // 128                # 16 row-tiles per block
CHUNK = int(os.environ.get("KMEANS_CHUNK", "512"))  # centroid chunk
KC = K // CHUNK
DC = D // 128                   # 8 contraction chunks

_compiled = {}


def _build(mode: str):
    """Build + compile the per-core Bass program. Returns the Bass object."""
    from contextlib import ExitStack
    import concourse.bacc as bacc
    import concourse.mybir as mybir
    import concourse.tile as tile

    f32 = mybir.dt.float32
    f32r = mybir.dt.float32r
    bf16 = mybir.dt.bfloat16
    u32 = mybir.dt.uint32

    nc = bacc.Bacc("TRN2", target_bir_lowering=False, debug=False)

    if mode == "f32r":
        mm_dt = f32r
        xt_d = [nc.dram_tensor("xt", [D, NC], f32r, kind="ExternalInput").ap()]
        c_d = [nc.dram_tensor("cent", [D, K], f32r, kind="ExternalInput").ap()]
        NMAT = [(0, 0)]
    else:  # bf16x2: hi/lo split, 3 matmuls
        mm_dt = bf16
        xt_d = [nc.dram_tensor(f"xt{i}", [D, NC], bf16, kind="ExternalInput").ap()
                for i in range(2)]
        c_d = [nc.dram_tensor(f"cent{i}", [D, K], bf16, kind="ExternalInput").ap()
               for i in range(2)]
        NMAT = [(0, 0), (0, 1), (1, 0)]
    nin = len(xt_d)

    bias_d = nc.dram_tensor("bias", [128, K], f32, kind="ExternalInput").ap()
    outv_d = nc.dram_tensor("outv", [NB, 128, NT * KC * 8], f32,
                            kind="ExternalOutput").ap()
    outi_d = nc.dram_tensor("outi", [NB, 128, NT * KC * 8], u32,
                            kind="ExternalOutput").ap()

    with tile.TileContext(nc) as tc:
        with ExitStack() as ctx:
            const_pool = ctx.enter_context(tc.tile_pool(name="const", bufs=1))
            xt_pool = ctx.enter_context(tc.tile_pool(name="xt", bufs=1))
            c_pool = ctx.enter_context(tc.tile_pool(name="cent", bufs=2))
            sc_pool = ctx.enter_context(tc.tile_pool(name="scores", bufs=4))
            acc_pool = ctx.enter_context(tc.tile_pool(name="acc", bufs=2))
            ps_pool = ctx.enter_context(tc.tile_pool(name="psum", bufs=(4 if CHUNK == 512 else 3), space="PSUM"))

            bias_sb = const_pool.tile([128, K], f32, name="bias_sb")
            nc.sync.dma_start(bias_sb[:], bias_d[:])

            for b in range(NB):
                # load x^T block: DC chunks of [128, NBLK] per input part
                xt_sb = [xt_pool.tile([128, DC * NBLK], mm_dt, name=f"xt_sb{i}",
                                      tag=f"xt{i}") for i in range(nin)]
                for i in range(nin):
                    for d in range(DC):
                        nc.sync.dma_start(
                            xt_sb[i][:, d * NBLK:(d + 1) * NBLK],
                            xt_d[i][d * 128:(d + 1) * 128, b * NBLK:(b + 1) * NBLK])

                mv_all = acc_pool.tile([128, NT * KC * 8], f32, name="mv_all", tag="mv")
                mi_all = acc_pool.tile([128, NT * KC * 8], u32, name="mi_all", tag="mi")

                for kc in range(KC):
                    c_sb = [c_pool.tile([128, DC * CHUNK], mm_dt, name=f"c_sb{i}",
                                        tag=f"c{i}") for i in range(nin)]
                    for i in range(nin):
                        for d in range(DC):
                            nc.sync.dma_start(
                                c_sb[i][:, d * CHUNK:(d + 1) * CHUNK],
                                c_d[i][d * 128:(d + 1) * 128,
                                       kc * CHUNK:(kc + 1) * CHUNK])
                    NSUB = CHUNK // 512
                    for nt in range(NT):
                        ps = ps_pool.tile([128, CHUNK], f32, name="ps")
                        nmm = len(NMAT) * DC
                        for d in range(DC):
                            for (ix, ic) in NMAT:
                                for j in range(NSUB):
                                    nc.tensor.matmul(
                                        ps[:, j * 512:(j + 1) * 512],
                                        xt_sb[ix][:, d * NBLK + nt * 128:
                                                  d * NBLK + (nt + 1) * 128],
                                        c_sb[ic][:, d * CHUNK + j * 512:
                                                 d * CHUNK + (j + 1) * 512],
                                        start=(d == 0 and (ix, ic) == NMAT[0]),
                                        stop=(d == DC - 1 and (ix, ic) == NMAT[-1]))
                        sc = sc_pool.tile([128, CHUNK], f32, name="sc")
                        nc.vector.tensor_tensor(
                            sc[:], ps[:], bias_sb[:, kc * CHUNK:(kc + 1) * CHUNK],
                            mybir.AluOpType.add)
                        col = nt * KC * 8 + kc * 8
                        nc.vector.max(mv_all[:, col:col + 8], sc[:])
                        nc.vector.max_index(mi_all[:, col:col + 8],
                                            mv_all[:, col:col + 8], sc[:])

                nc.sync.dma_start(outv_d[b], mv_all[:])
                nc.sync.dma_start(outi_d[b], mi_all[:])
    nc.compile()
    return nc


def _get_nc(mode: str):
    if mode not in _compiled:
        _compiled[mode] = _build(mode)
    return _compiled[mode]


def _merge_host(outv, outi):
    """Merge per-chunk top-1 candidates -> global argmax indices [NC]."""
    # outv/outi: [NB, 128, NT*KC*8]
    vals = outv.reshape(NB, 128, NT, KC, 8).transpose(0, 2, 1, 3, 4)
    idxs = outi.reshape(NB, 128, NT, KC, 8).transpose(0, 2, 1, 3, 4)
    vals = vals.reshape(NC, KC, 8)
    idxs = idxs.reshape(NC, KC, 8)
    v0 = vals[:, :, 0]
    i0 = idxs[:, :, 0].astype(np.int64)
    am = np.argmax(v0, axis=1)            # first occurrence on ties
    rows = np.arange(NC)
    gi = am * CHUNK + i0[rows, am]
    return gi.astype(np.int32), vals, idxs


def kernel(x: np.ndarray, centroids: np.ndarray) -> np.ndarray:
    mode = os.environ.get("KMEANS_MM_MODE", "f32r")
    refine = int(os.environ.get("KMEANS_REFINE", "1"))
    from concourse.bass_utils import run_bass_kernel_spmd

    x = np.asarray(x, dtype=np.float32)
    centroids = np.asarray(centroids, dtype=np.float32)
    nc = _get_nc(mode)

    xt = np.ascontiguousarray(x.T)                       # [D, N]
    bias_row = -0.5 * np.einsum("dk,dk->k", centroids, centroids,
                                dtype=np.float64).astype(np.float32)
    bias = np.ascontiguousarray(np.broadcast_to(bias_row, (128, K)))

    in_maps = []
    for c in range(N_CORES):
        sl = np.ascontiguousarray(xt[:, c * NC:(c + 1) * NC])
        if mode == "f32r":
            m = {"xt": sl, "cent": centroids, "bias": bias}
        else:
            import ml_dtypes
            xh = sl.astype(ml_dtypes.bfloat16)
            xl = (sl - xh.astype(np.float32)).astype(ml_dtypes.bfloat16)
            ch = centroids.astype(ml_dtypes.bfloat16)
            cl = (centroids - ch.astype(np.float32)).astype(ml_dtypes.bfloat16)
            m = {"xt0": xh, "xt1": xl, "cent0": ch, "cent1": cl, "bias": bias}
        in_maps.append(m)

    res = run_bass_kernel_spmd(nc, in_maps, core_ids=list(range(N_CORES)))

    out = np.empty(N_FULL, dtype=np.int32)
    for c in range(N_CORES):
        gi, vals, idxs = _merge_host(res.results[c]["outv"], res.results[c]["outi"])
        if refine:
            gi = _refine(x[c * NC:(c + 1) * NC], centroids, bias_row, vals, idxs)
        out[c * NC:(c + 1) * NC] = gi
    return out


def _refine(xs, centroids, bias_row, vals, idxs, top=8):
    """Re-score each row's top candidates exactly in fp32 to undo f32r rounding."""
    n = xs.shape[0]
    fv = vals.reshape(n, KC * 8)
    fi = (idxs.astype(np.int64)
          + (np.arange(KC) * CHUNK)[None, :, None]).reshape(n, KC * 8)
    part = np.argpartition(-fv, top - 1, axis=1)[:, :top]
    cand = np.take_along_axis(fi, part, axis=1)          # [n, top] global idx
    # exact scores for candidates, batched
    out = np.empty(n, dtype=np.int32)
    bs = 4096
    for s in range(0, n, bs):
        e = min(s + bs, n)
        cb = cand[s:e]                                   # [b, top]
        cc = centroids.T[cb]                             # [b, top, D]
        sc = np.einsum("bd,btd->bt", xs[s:e], cc, dtype=np.float64)
        sc = sc + bias_row[cb]
        # argmax with ties -> smallest global index (first occurrence in k)
        best = sc.max(axis=1, keepdims=True)
        big = np.where(sc >= best, cb, np.iinfo(np.int64).max)
        out[s:e] = big.min(axis=1).astype(np.int32)
    return out



# revision 2
# speedup vs baseline: 4.4185x; 4.4185x over previous
"""Trainium2 Bass kernel for KMeans assignment (argmin over 8192 centroids).

Problem: x [32768, 1024] f32, centroids [1024, 8192] f32 ->
         argmin_k ||x_n - c_k||^2  as int32 [32768].

Math: argmin_k (||x||^2 - 2 x.c_k + ||c_k||^2) == argmax_k (x.c_k - 0.5*||c_k||^2).
The ||x||^2 term is row-constant and drops out of the argmin.

Sharding: data-parallel over N across 8 cores (4096 rows each), centroids
replicated.

Per core: x and centroids are pre-quantized to fp8 e4m3 on the host; the
tensor engine runs DoubleRow fp8 matmuls (256-wide contraction per call,
2x PE throughput = ~157 TF/s) accumulating 2048-wide PSUM tiles. The DVE
adds the f32 bias (-0.5*||c_k||^2, computed exactly on host) and writes
f16 scores to SBUF; one max8 + max_index per 128-row tile over the full
K=8192 yields each row's global top-8 candidate values + indices. The
host then re-scores the top-8 candidates exactly in fp64 (undoing fp8
quantization noise, sigma~1.6 vs typical top-1/top-2 gap ~10) and picks
the argmax with first-index tie-breaking.
"""
import numpy as np

# ---- problem constants (hardcoded per harness contract) ----
N_FULL, D, K = 32768, 1024, 8192
N_CORES = 8
NC = N_FULL // N_CORES          # 4096 rows per core
T = NC // 128                   # 32 row-tiles per core
RB = 4                          # row-tiles per row block
NRB = T // RB                   # 8 row blocks
KQ = 4                          # K quarters
KQW = K // KQ                   # 2048 centroids per quarter
DC = 4                          # 256-wide contraction chunks (DoubleRow)

_compiled = {}


def _build(reps: int = 1):
    """Build + compile the per-core Bass program (body unrolled `reps`
    times; reps>1 exists only for slope-based timing)."""
    from contextlib import ExitStack
    import concourse.bacc as bacc
    import concourse.mybir as mybir
    import concourse.tile as tile

    f32 = mybir.dt.float32
    f16 = mybir.dt.float16
    f8 = mybir.dt.float8e4
    u16 = mybir.dt.uint16
    DR = mybir.MatmulPerfMode.DoubleRow

    nc = bacc.Bacc("TRN2", target_bir_lowering=False, debug=False)

    xt_d = nc.dram_tensor("xt", [D, NC], f8, kind="ExternalInput").ap()
    c_d = nc.dram_tensor("cent", [D, K], f8, kind="ExternalInput").ap()
    bias_d = nc.dram_tensor("bias", [128, K], f32, kind="ExternalInput").ap()
    outv_d = nc.dram_tensor("outv", [128, T * 8], f16, kind="ExternalOutput").ap()
    outi_d = nc.dram_tensor("outi", [128, T * 8], u16, kind="ExternalOutput").ap()

    with tile.TileContext(nc) as tc:
        with ExitStack() as ctx:
            const_pool = ctx.enter_context(tc.tile_pool(name="const", bufs=1))
            xt_pool = ctx.enter_context(tc.tile_pool(name="xt", bufs=1))
            c_pool = ctx.enter_context(tc.tile_pool(name="cent", bufs=2))
            sc_pool = ctx.enter_context(tc.tile_pool(name="scores", bufs=1))
            out_pool = ctx.enter_context(tc.tile_pool(name="out", bufs=1))
            ps_pool = ctx.enter_context(
                tc.tile_pool(name="psum", bufs=2, space="PSUM"))

            for _ in range(reps):
                bias_sb = const_pool.tile([128, K], f32, name="bias_sb",
                                          tag="bias")
                nc.sync.dma_start(bias_sb[:], bias_d[:])

                xt_sb = [xt_pool.tile([128, 2, NC], f8, name=f"xt_sb{dc}",
                                      tag=f"xt{dc}") for dc in range(DC)]
                for dc in range(DC):
                    for i in range(2):
                        r0 = dc * 256 + i * 128
                        nc.sync.dma_start(xt_sb[dc][:, i, :],
                                          xt_d[r0:r0 + 128, :])

                mv_all = out_pool.tile([128, T * 8], f16, name="mv", tag="mv")
                mi_all = out_pool.tile([128, T * 8], u16, name="mi", tag="mi")

                for rb in range(NRB):
                    sc_t = [sc_pool.tile([128, K], f16, name=f"sc{nt}",
                                         tag=f"sc{nt}") for nt in range(RB)]
                    for kq in range(KQ):
                        c_sb = [c_pool.tile([128, 2, KQW], f8, name=f"c_sb{dc}",
                                            tag=f"c{dc}") for dc in range(DC)]
                        for dc in range(DC):
                            for i in range(2):
                                r0 = dc * 256 + i * 128
                                nc.sync.dma_start(
                                    c_sb[dc][:, i, :],
                                    c_d[r0:r0 + 128, kq * KQW:(kq + 1) * KQW])
                        for nt in range(RB):
                            t = rb * RB + nt
                            ps = ps_pool.tile([128, KQW], f32, name="ps",
                                              tag="ps")
                            for dc in range(DC):
                                for j in range(KQW // 512):
                                    nc.tensor.matmul(
                                        ps[:, j * 512:(j + 1) * 512],
                                        xt_sb[dc][:, :, t * 128:(t + 1) * 128],
                                        c_sb[dc][:, :, j * 512:(j + 1) * 512],
                                        start=(dc == 0), stop=(dc == DC - 1),
                                        perf_mode=DR)
                            nc.vector.tensor_tensor(
                                sc_t[nt][:, kq * KQW:(kq + 1) * KQW],
                                ps[:], bias_sb[:, kq * KQW:(kq + 1) * KQW],
                                mybir.AluOpType.add)
                    for nt in range(RB):
                        t = rb * RB + nt
                        nc.vector.max(mv_all[:, t * 8:(t + 1) * 8], sc_t[nt][:])
                        nc.vector.max_index(mi_all[:, t * 8:(t + 1) * 8],
                                            mv_all[:, t * 8:(t + 1) * 8],
                                            sc_t[nt][:])

                nc.sync.dma_start(outv_d[:], mv_all[:])
                nc.sync.dma_start(outi_d[:], mi_all[:])
    nc.compile()
    return nc


def _get_nc(reps: int = 1):
    if reps not in _compiled:
        _compiled[reps] = _build(reps)
    return _compiled[reps]


def _prepare_in_maps(x: np.ndarray, centroids: np.ndarray):
    """Host-side prep shared by kernel() and the timing harness."""
    import ml_dtypes

    f8 = ml_dtypes.float8_e4m3
    xt8 = np.ascontiguousarray(x.T).astype(f8)              # [D, N]
    c8 = np.ascontiguousarray(centroids.astype(f8))          # [D, K]
    bias_row = -0.5 * np.einsum("dk,dk->k", centroids, centroids,
                                dtype=np.float64).astype(np.float32)
    bias = np.ascontiguousarray(np.broadcast_to(bias_row, (128, K)))
    in_maps = []
    for c in range(N_CORES):
        in_maps.append({
            "xt": np.ascontiguousarray(xt8[:, c * NC:(c + 1) * NC]),
            "cent": c8,
            "bias": bias,
        })
    return in_maps


def _candidates(outv: np.ndarray, outi: np.ndarray):
    """Device outputs -> per-row candidate (values, indices) [NC, 8]."""
    vals = outv.reshape(128, T, 8).transpose(1, 0, 2).reshape(NC, 8)
    idxs = outi.reshape(128, T, 8).transpose(1, 0, 2).reshape(NC, 8)
    return vals.astype(np.float32), idxs.astype(np.int64)


def _refine(xs, centroids, bias_row, cand):
    """Exact fp64 rescore of each row's candidates; argmax with
    first-index tie-break (matches reference argmin semantics)."""
    n = xs.shape[0]
    cand = np.clip(cand, 0, K - 1)                      # guard vs sentinel
    out = np.empty(n, dtype=np.int64)
    bs = 4096
    cT = centroids.T                                     # [K, D]
    for s in range(0, n, bs):
        e = min(s + bs, n)
        cb = cand[s:e]                                   # [b, 8]
        cc = cT[cb]                                      # [b, 8, D]
        sc = np.einsum("bd,btd->bt", xs[s:e].astype(np.float64), cc,
                       dtype=np.float64)
        sc = sc + bias_row[cb]
        best = sc.max(axis=1, keepdims=True)
        big = np.where(sc >= best, cb, np.iinfo(np.int64).max)
        out[s:e] = big.min(axis=1)
    return out.astype(np.int32)


def kernel(x: np.ndarray, centroids: np.ndarray) -> np.ndarray:
    from concourse.bass_utils import run_bass_kernel_spmd

    x = np.asarray(x, dtype=np.float32)
    centroids = np.asarray(centroids, dtype=np.float32)
    nc = _get_nc()
    in_maps = _prepare_in_maps(x, centroids)
    res = run_bass_kernel_spmd(nc, in_maps, core_ids=list(range(N_CORES)))

    bias_row = -0.5 * np.einsum("dk,dk->k", centroids, centroids,
                                dtype=np.float64)
    out = np.empty(N_FULL, dtype=np.int32)
    for c in range(N_CORES):
        vals, idxs = _candidates(res.results[c]["outv"],
                                 res.results[c]["outi"])
        out[c * NC:(c + 1) * NC] = _refine(
            x[c * NC:(c + 1) * NC], centroids, bias_row, idxs)
    return out
